# revision 11
# baseline (speedup 1.0000x reference)
"""GNN message-passing convolution on 8 Trainium2 NeuronCores (v2).

Strategy (receiver-sharded, zero collectives):
  - Host sorts edges by receiver; core k owns receivers [6250k, 6250(k+1)).
  - Windows of 128 receivers; per-window variable chunk counts (max over
    cores so one program serves all 8 cores). Chunks of 128 edge slots,
    lo (sender < 32768) then hi, batched WB windows per dma_gather pair.
  - The edge MLP (mix gates = F(ea0), a 1-D function) is replaced by a
    sigmoid-basis fit: hpw_k = sigmoid(a_k*ea0 + b_k) via one K-unit
    matmul + ACT sigmoid, then mix = hpw^T @ Wc per chunk. Pad slots use
    an ea0 far left of every knot so all gates underflow to zero.
  - DVE does the equivariant tensor product + gating in bf16 with layouts
    chosen so every op qualifies for the 2x_1p mode; the receiver one-hot
    uses an iota table with the broadcast on a middle dim for the same
    reason. ea1 replication runs on GpSimd, mix PSUM->SBUF copies on ACT.
  - Scatter-add via one-hot matmul into per-window PSUM accumulators;
    pipelined emission (production / DVE / scatter skewed by 2 groups)
    keeps the PE streaming continuously at its full 2.4 GHz p-state.
  - Output flushed in bf16; host concatenates, casts, un-permutes.
"""

import numpy as np

N_NODES = 50000
N_EDGES = 800000
MUL = 32
NCORES = 8
NODES_PER_CORE = N_NODES // NCORES          # 6250
P = 128
WINDOWS = (NODES_PER_CORE + P - 1) // P     # 49
OUT_ROWS = WINDOWS * P                      # 6272
SPLIT = 32768                               # int16 index limit
INV_SQRT3 = 1.0 / np.sqrt(3.0)
AVG_NUM_NEIGHBORS = 16.0
WB = 4                                      # windows per gather batch
CGMAX = 24                                  # chunks per compute group
KNOTS = 64

_CACHE = {}
_DEBUG_DUMP = False


def _col_perms():
    # node table planar permutation: new[32+32*i+c] = old[32+3*c+i]
    node_perm = np.concatenate(
        [np.arange(32)]
        + [np.array([32 + 3 * c + i for c in range(32)]) for i in range(3)]
    )
    # output un-permutation: ref[64+3c+i] = int[64+32i+c]; same at 160
    out_perm = np.empty(256, np.int64)
    out_perm[0:64] = np.arange(64)
    for c in range(32):
        for i in range(3):
            out_perm[64 + 3 * c + i] = 64 + 32 * i + c
            out_perm[160 + 3 * c + i] = 160 + 32 * i + c
    return node_perm, out_perm


def _silu(x):
    return x / (1.0 + np.exp(-x))


def _sigmoid(z):
    return 1.0 / (1.0 + np.exp(-np.clip(z, -60, 60)))


def _fit_pwl(w_mlp0, w_mlp1, w_mlp2, xs):
    """Fit mix = F(ea0) (the whole gate MLP incl. scalings) with a K-unit
    sigmoid basis: hpw_k = sigmoid(a_k*x + b_k). Coefficients are
    sigma-delta quantized to bf16 so prefix-sum rounding error stays at
    one quantum. Returns (wA [2,K] f32 on the bf16 grid, Wc [K,128] f32
    on the bf16 grid, pad value, validation rel err)."""
    import ml_dtypes
    bf = ml_dtypes.bfloat16

    w0 = np.asarray(w_mlp0, np.float64)
    w1 = np.asarray(w_mlp1, np.float64)
    w2s = np.asarray(w_mlp2, np.float64) / 32.0
    w2s = w2s.copy()
    w2s[:, 32:64] *= INV_SQRT3

    def F(x):
        h = _silu(x[:, None] * w0[0][None, :])
        h = _silu((h @ w1) * 0.125)
        return h @ w2s

    xs = np.asarray(xs, np.float64)
    xmin, xmax = xs.min(), xs.max()
    span = xmax - xmin
    t = xmin - 0.02 * span + (1.04 * span) * np.arange(KNOTS) / (KNOTS - 1)
    tau = 0.7 * (t[1] - t[0])
    inv_tau = float(np.float32(1.0 / tau))
    a = np.full(KNOTS, inv_tau, np.float32).astype(bf).astype(np.float64)
    b = (-t * inv_tau).astype(np.float32).astype(bf).astype(np.float64)

    qs = np.quantile(xs, np.linspace(0.0, 1.0, 8193))
    xg = np.concatenate([np.linspace(xmin, xmax, 4096), qs])
    B = _sigmoid(xg[:, None] * a[None, :] + b[None, :])
    lam = 1e-5 * len(xg)
    c = np.linalg.solve(B.T @ B + lam * np.eye(KNOTS), B.T @ F(xg))

    # sigma-delta (error-feedback) bf16 quantization along k
    cq = np.zeros_like(c)
    e = np.zeros(c.shape[1])
    for k in range(KNOTS):
        v = c[k] + e
        q = np.asarray(v, np.float32).astype(bf).astype(np.float64)
        cq[k] = q
        e = v - q

    # pad value: all basis outputs ~0 -> all gates ~0
    pad_val = float(np.float32(t[0] - 80.0 * tau))

    # validate with device-like arithmetic on a subsample
    sub = xs[:: max(1, len(xs) // 100000)]
    xsb = sub.astype(np.float32).astype(bf).astype(np.float64)
    hp = _sigmoid(xsb[:, None] * a + b).astype(np.float32).astype(bf) \
        .astype(np.float64)
    F_x = F(sub)
    err = np.linalg.norm(hp @ cq - F_x) / np.linalg.norm(F_x)

    wA = np.stack([a, b]).astype(np.float32)
    return wA, cq.astype(np.float32), pad_val, err


def _wrap_idx(a):
    """[n] int16 -> [128, n/16] wrapped (flat i at [i%16, i//16], x8)."""
    n = a.shape[0]
    w = a.reshape(n // 16, 16).T            # [16, n/16]
    return np.ascontiguousarray(np.tile(w, (8, 1)))


class _Chunk:
    __slots__ = ("win", "start", "stop")

    def __init__(self, win, start, stop):
        self.win = win
        self.start = start
        self.stop = stop


class _Group:
    __slots__ = ("batch", "c0", "cg", "first_in_batch")

    def __init__(self, batch, c0, cg, first_in_batch):
        self.batch = batch
        self.c0 = c0            # global chunk index
        self.cg = cg
        self.first_in_batch = first_in_batch


class _Batch:
    __slots__ = ("windows", "CLb", "CHb", "Cb", "coff")

    def __init__(self, windows, CLb, CHb, Cb, coff):
        self.windows = windows
        self.CLb = CLb
        self.CHb = CHb
        self.Cb = Cb
        self.coff = coff        # global chunk offset


def _build_structure(nlo, nhi):
    """nlo/nhi: [NCORES, WINDOWS] edge counts. Returns shared structure."""
    CL = np.maximum((nlo.max(axis=0) + P - 1) // P, 0).astype(np.int64)
    CH = np.maximum((nhi.max(axis=0) + P - 1) // P, 0).astype(np.int64)
    assert np.all(CL + CH >= 1)

    batches = []
    chunks = []
    lo_base = np.zeros(WINDOWS, np.int64)   # global chunk idx of window's lo run
    hi_base = np.zeros(WINDOWS, np.int64)
    coff = 0
    for b0 in range(0, WINDOWS, WB):
        wl = list(range(b0, min(b0 + WB, WINDOWS)))
        CLb = int(sum(CL[w] for w in wl))
        CHb = int(sum(CH[w] for w in wl))
        batches.append(_Batch(wl, CLb, CHb, CLb + CHb, coff))
        c = coff
        for w in wl:
            lo_base[w] = c
            for j in range(CL[w]):
                chunks.append(_Chunk(w, j == 0, CH[w] == 0 and j == CL[w] - 1))
            c += CL[w]
        for w in wl:
            hi_base[w] = c
            for j in range(CH[w]):
                chunks.append(_Chunk(w, CL[w] == 0 and j == 0, j == CH[w] - 1))
            c += CH[w]
        coff = c
    TC = coff

    groups = []
    for bi, bt in enumerate(batches):
        ng = (bt.Cb + CGMAX - 1) // CGMAX
        sizes = [bt.Cb // ng + (1 if i < bt.Cb % ng else 0) for i in range(ng)]
        c = bt.coff
        for i, sz in enumerate(sizes):
            groups.append(_Group(bi, c, sz, i == 0))
            c += sz
    return CL, CH, batches, chunks, groups, lo_base, hi_base, TC


def _build_program(meta):
    import concourse.bacc as bacc
    import concourse.bass as bass  # noqa: F401
    import concourse.mybir as mybir
    import concourse.tile as tile

    f32 = mybir.dt.float32
    bf16 = mybir.dt.bfloat16
    i16 = mybir.dt.int16
    AF = mybir.ActivationFunctionType
    OP = mybir.AluOpType

    CL, CH, batches, chunks, groups, lo_base, hi_base, TC = meta
    nb = len(batches)
    ng = len(groups)
    CBMAX = max(bt.Cb for bt in batches)
    K = KNOTS

    nc = bacc.Bacc("TRN2", target_bir_lowering=False, debug=False,
                   num_devices=NCORES, num_swdge_queues=4)

    node_d = nc.dram_tensor("node_bf", [N_NODES, 128], bf16, kind="ExternalInput")
    lo_ds, hi_ds = [], []
    for b, bt in enumerate(batches):
        lo_ds.append(nc.dram_tensor(f"lo_i{b}", [P, max(bt.CLb * P // 16, 1)],
                                    i16, kind="ExternalInput"))
        hi_ds.append(nc.dram_tensor(f"hi_i{b}", [P, max(bt.CHb * P // 16, 1)],
                                    i16, kind="ExternalInput"))
    ea4_d = nc.dram_tensor("ea4", [P, TC, 4], bf16, kind="ExternalInput")
    rcv_d = nc.dram_tensor("rcv_f", [P, TC], bf16, kind="ExternalInput")
    ea0_d = nc.dram_tensor("ea0r2", [2, TC * P], bf16, kind="ExternalInput")
    wA_d = nc.dram_tensor("wA", [2, K], bf16, kind="ExternalInput")
    wc_d = nc.dram_tensor("wc_dup", [128, 128], bf16, kind="ExternalInput")
    iota_d = nc.dram_tensor("iota_f", [P, P, CGMAX], bf16, kind="ExternalInput")
    out_d = nc.dram_tensor("out", [OUT_ROWS, 256], bf16, kind="ExternalOutput")

    dbg = {}
    if _DEBUG_DUMP:
        dbg["G"] = nc.dram_tensor("dbg_G", [P, CGMAX, 128], bf16,
                                  kind="ExternalOutput")
        dbg["hpw"] = nc.dram_tensor("dbg_hpw", [128, 3 * 512], bf16,
                                    kind="ExternalOutput")
        dbg["mix"] = nc.dram_tensor("dbg_mix", [P, CGMAX, 128], bf16,
                                    kind="ExternalOutput")
        dbg["msgs"] = nc.dram_tensor("dbg_msgs", [P, CGMAX, 256], bf16,
                                     kind="ExternalOutput")
        dbg["oh"] = nc.dram_tensor("dbg_oh", [P, P, CGMAX], bf16,
                                   kind="ExternalOutput")
        dbg["earep"] = nc.dram_tensor("dbg_earep", [P, CGMAX, 3, 32], bf16,
                                      kind="ExternalOutput")

    node_ap = node_d.ap()
    node_lo = node_ap[0:SPLIT, :]
    node_hi = node_ap[SPLIT:N_NODES, :]

    with tile.TileContext(nc) as tc:
        with (
            tc.tile_pool(name="const", bufs=1) as cp,
            tc.tile_pool(name="gpool", bufs=2) as gp,
            tc.tile_pool(name="bload", bufs=2) as bp,
            tc.tile_pool(name="sb", bufs=2) as sb,
            tc.tile_pool(name="ea0p", bufs=3) as e0p_,
            tc.tile_pool(name="stage", bufs=2) as stp,
            tc.tile_pool(name="psH", bufs=2, space="PSUM") as psH,
            tc.tile_pool(name="psM", bufs=2, space="PSUM") as psM,
            tc.tile_pool(name="psA", bufs=4, space="PSUM") as psA,
        ):
            # ---- resident constants ----
            wA_t = cp.tile([2, K], bf16)
            nc.sync.dma_start(out=wA_t[:], in_=wA_d.ap())
            wc_t = cp.tile([128, 128], bf16)
            nc.sync.dma_start(out=wc_t[:], in_=wc_d.ap())
            iota_t = cp.tile([P, P, CGMAX], bf16)
            nc.sync.dma_start(out=iota_t[:], in_=iota_d.ap())

            # per-batch tiles, filled by emit_batch_load
            bstate = {}

            def emit_batch_load(b):
                bt = batches[b]
                G = gp.tile([P, CBMAX, 128], bf16, tag="G", name=f"G_b{b}")
                ea4b = bp.tile([P, CBMAX, 4], bf16, tag="ea4", name=f"ea4_b{b}")
                nc.sync.dma_start(out=ea4b[:, 0:bt.Cb, :],
                                  in_=ea4_d.ap()[:, bt.coff:bt.coff + bt.Cb, :])
                rcvb = bp.tile([P, CBMAX], bf16, tag="rcv", name=f"rcv_b{b}")
                nc.sync.dma_start(out=rcvb[:, 0:bt.Cb],
                                  in_=rcv_d.ap()[:, bt.coff:bt.coff + bt.Cb])
                if bt.CLb > 0:
                    li = bp.tile([P, max(CBMAX * 8, 1)], i16, tag="li",
                                 name=f"li_b{b}")
                    nc.sync.dma_start(out=li[:, 0:bt.CLb * 8], in_=lo_ds[b].ap())
                    nc.gpsimd.dma_gather(
                        G[:, 0:bt.CLb, :], node_lo, li[:, 0:bt.CLb * 8],
                        bt.CLb * P, bt.CLb * P, 128,
                        single_packet=False, queue_num=(2 * b) % 4)
                if bt.CHb > 0:
                    hi = bp.tile([P, max(CBMAX * 8, 1)], i16, tag="hi",
                                 name=f"hi_b{b}")
                    nc.sync.dma_start(out=hi[:, 0:bt.CHb * 8], in_=hi_ds[b].ap())
                    nc.gpsimd.dma_gather(
                        G[:, bt.CLb:bt.Cb, :], node_hi, hi[:, 0:bt.CHb * 8],
                        bt.CHb * P, bt.CHb * P, 128,
                        single_packet=False, queue_num=(2 * b + 1) % 4)
                bstate[b] = (G, ea4b, rcvb)

            def emit_ea0(s):
                g = groups[s]
                ne = g.cg * P
                t = e0p_.tile([2, CGMAX * P], bf16, tag="ea0", name=f"ea0_s{s}")
                nc.sync.dma_start(
                    out=t[:, 0:ne],
                    in_=ea0_d.ap()[:, g.c0 * P:g.c0 * P + ne])
                return t

            def emit_prod(s, ea0t):
                """hpw matmuls + relu + mix matmuls + mix copies for group s."""
                g = groups[s]
                ne = g.cg * P
                ntile = (ne + 1023) // 1024
                hpws = []
                for t in range(ntile):
                    lo0 = t * 1024
                    cols_lo = min(512, ne - lo0)
                    cols_hi = max(0, min(512, ne - lo0 - 512))
                    hp = psH.tile([128, 512], f32, tag="hpw",
                                  name=f"hpw_s{s}_{t}")
                    nc.tensor.matmul(out=hp[0:64, 0:cols_lo], lhsT=wA_t[:, :],
                                     rhs=ea0t[:, lo0:lo0 + cols_lo],
                                     start=True, stop=True,
                                     tile_position=(0, 0))
                    if cols_hi > 0:
                        nc.tensor.matmul(out=hp[64:128, 0:cols_hi],
                                         lhsT=wA_t[:, :],
                                         rhs=ea0t[:, lo0 + 512:lo0 + 512 + cols_hi],
                                         start=True, stop=True,
                                         tile_position=(0, 64))
                    hs = sb.tile([128, 512], bf16, tag="hpwsb",
                                 name=f"hpwsb_s{s}_{t}")
                    if cols_hi == cols_lo:
                        nc.scalar.activation(out=hs[:, 0:cols_lo],
                                             in_=hp[:, 0:cols_lo], func=AF.Sigmoid)
                    else:
                        nc.scalar.activation(out=hs[0:64, 0:cols_lo],
                                             in_=hp[0:64, 0:cols_lo], func=AF.Sigmoid)
                        if cols_hi > 0:
                            nc.scalar.activation(out=hs[64:128, 0:cols_hi],
                                                 in_=hp[64:128, 0:cols_hi],
                                                 func=AF.Sigmoid)
                    hpws.append(hs)

                mix_sb = sb.tile([P, CGMAX, 128], bf16, tag="mix",
                                 name=f"mix_s{s}")
                for q0 in range(0, g.cg, 4):
                    qn = min(4, g.cg - q0)
                    mp = psM.tile([128, 4, 128], f32, tag="mixp",
                                  name=f"mixp_s{s}_{q0}")
                    for jj in range(qn):
                        c = q0 + jj
                        t = c // 8
                        half = (c % 8) // 4
                        col = (c % 4) * P
                        nc.tensor.matmul(
                            out=mp[:, jj, :],
                            lhsT=hpws[t][half * 64:half * 64 + 64, col:col + P],
                            rhs=wc_t[half * 64:half * 64 + 64, :],
                            start=True, stop=True,
                            tile_position=(half * 64, 0))
                    nc.scalar.activation(out=mix_sb[:, q0:q0 + qn, :],
                                         in_=mp[:, 0:qn, :], func=AF.Copy)
                if _DEBUG_DUMP and s == 0:
                    for t, hs in enumerate(hpws):
                        nc.sync.dma_start(
                            out=dbg["hpw"].ap()[:, t * 512:(t + 1) * 512],
                            in_=hs[:, :])
                    nc.sync.dma_start(out=dbg["mix"].ap()[:, 0:g.cg, :],
                                      in_=mix_sb[:, 0:g.cg, :])
                return mix_sb

            def emit_ea_rep(s):
                g = groups[s]
                _, ea4b, _ = bstate[g.batch]
                cb0 = g.c0 - batches[g.batch].coff
                er = sb.tile([P, CGMAX, 3, 32], bf16, tag="earep",
                             name=f"earep_s{s}")
                src = ea4b[:, cb0:cb0 + g.cg, 0:3].unsqueeze(3) \
                    .to_broadcast([P, g.cg, 3, 32])
                nc.gpsimd.tensor_copy(out=er[:, 0:g.cg, :, :], in_=src)
                if _DEBUG_DUMP and s == 0:
                    nc.sync.dma_start(out=dbg["earep"].ap()[:, 0:g.cg, :, :],
                                      in_=er[:, 0:g.cg, :, :])
                return er

            def emit_dve(s, mix_sb, er):
                g = groups[s]
                G, _, rcvb = bstate[g.batch]
                cb0 = g.c0 - batches[g.batch].coff
                cg = g.cg
                Gg = G[:, cb0:cb0 + cg, :]
                Gv = Gg[:, :, 32:128].rearrange("p g (i c) -> p g i c", i=3)
                Gs = Gg[:, :, 0:32]
                erg = er[:, 0:cg, :, :]

                msgs = sb.tile([P, CGMAX, 256], bf16, tag="msgs",
                               name=f"msgs_s{s}")
                T96 = msgs[:, 0:cg, 160:256].rearrange(
                    "p g (i c) -> p g i c", i=3)
                tp0 = sb.tile([P, CGMAX, 32], bf16, tag="tp0", name=f"tp0_s{s}")
                tpf = sb.tile([P, CGMAX, 32], bf16, tag="tpf", name=f"tpf_s{s}")
                sg2 = sb.tile([P, CGMAX, 32], bf16, tag="sg2", name=f"sg2_s{s}")
                oh = sb.tile([P, P, CGMAX], bf16, tag="oh", name=f"oh_s{s}")

                # tp path (tmp96 lives in the msgs 160:256 region)
                nc.vector.tensor_tensor(out=T96, in0=Gv, in1=erg, op=OP.mult)
                nc.vector.tensor_tensor(out=tp0[:, 0:cg, :],
                                        in0=msgs[:, 0:cg, 160:192],
                                        in1=msgs[:, 0:cg, 192:224], op=OP.add)
                nc.vector.tensor_tensor(out=tpf[:, 0:cg, :],
                                        in0=tp0[:, 0:cg, :],
                                        in1=msgs[:, 0:cg, 224:256], op=OP.add)
                # receiver one-hot: iota packed last, rcv broadcast on mid dim
                rb = rcvb[:, cb0:cb0 + cg].unsqueeze(1).to_broadcast([P, P, cg])
                nc.vector.tensor_tensor(out=oh[:, :, 0:cg],
                                        in0=iota_t[:, :, 0:cg], in1=rb,
                                        op=OP.is_equal)
                # gating
                nc.vector.tensor_tensor(out=msgs[:, 0:cg, 0:32], in0=Gs,
                                        in1=mix_sb[:, 0:cg, 0:32], op=OP.mult)
                nc.vector.tensor_tensor(out=msgs[:, 0:cg, 32:64],
                                        in0=tpf[:, 0:cg, :],
                                        in1=mix_sb[:, 0:cg, 32:64], op=OP.mult)
                mv = mix_sb[:, 0:cg, 64:96].unsqueeze(2) \
                    .to_broadcast([P, cg, 3, 32])
                nc.vector.tensor_tensor(
                    out=msgs[:, 0:cg, 64:160].rearrange(
                        "p g (i c) -> p g i c", i=3),
                    in0=Gv, in1=mv, op=OP.mult)
                nc.vector.tensor_tensor(out=sg2[:, 0:cg, :], in0=Gs,
                                        in1=mix_sb[:, 0:cg, 96:128], op=OP.mult)
                sgb = sg2[:, 0:cg, :].unsqueeze(2).to_broadcast([P, cg, 3, 32])
                nc.vector.tensor_tensor(out=T96, in0=sgb, in1=erg, op=OP.mult)
                if _DEBUG_DUMP and s == 0:
                    nc.sync.dma_start(out=dbg["G"].ap()[:, 0:cg, :], in_=Gg)
                    nc.sync.dma_start(out=dbg["msgs"].ap()[:, 0:cg, :],
                                      in_=msgs[:, 0:cg, :])
                    nc.sync.dma_start(out=dbg["oh"].ap()[:, :, 0:cg],
                                      in_=oh[:, :, 0:cg])
                return msgs, oh

            open_accs = {}
            closed = {}

            def emit_scatter(s, msgs, oh):
                # acc tiles are [128, 512] f32 = exactly one PSUM bank each:
                # two OPEN accumulation groups must never share a bank.
                g = groups[s]
                for j in range(g.cg):
                    ck = chunks[g.c0 + j]
                    w = ck.win
                    if w not in open_accs:
                        open_accs[w] = psA.tile([128, 512], f32, tag="acc",
                                                name=f"acc_w{w}")
                    acc = open_accs[w]
                    nc.tensor.matmul(out=acc[:, 0:256],
                                     lhsT=oh[:, :, j], rhs=msgs[:, j, :],
                                     start=ck.start, stop=ck.stop)
                    if ck.stop:
                        acc = open_accs.pop(w)
                        ot = stp.tile([128, 256], bf16, tag="ot",
                                      name=f"ot_w{w}")
                        nc.scalar.activation(out=ot[:, :], in_=acc[:, 0:256],
                                             func=AF.Copy)
                        nc.sync.dma_start(
                            out=out_d.ap()[w * P:(w + 1) * P, :],
                            in_=ot[:, :])

            # ---- software pipeline: prod(s) / dve(s-1) / scatter(s-2) ----
            ea0_tiles = {}
            prod = {}
            dve = {}
            for it in range(ng + 2):
                if it < ng:
                    g = groups[it]
                    if g.first_in_batch:
                        if it == 0:
                            emit_batch_load(0)
                        if g.batch + 1 < nb:
                            emit_batch_load(g.batch + 1)
                    if it == 0:
                        ea0_tiles[0] = emit_ea0(0)
                    if it + 1 < ng:
                        ea0_tiles[it + 1] = emit_ea0(it + 1)
                    mix_sb = emit_prod(it, ea0_tiles.pop(it))
                    er = emit_ea_rep(it)
                    prod[it] = (mix_sb, er)
                if it - 1 >= 0 and it - 1 < ng:
                    mix_sb, er = prod.pop(it - 1)
                    dve[it - 1] = emit_dve(it - 1, mix_sb, er)
                if it - 2 >= 0:
                    msgs, oh = dve.pop(it - 2)
                    emit_scatter(it - 2, msgs, oh)

    nc.compile()
    return nc


def _prep_inputs(node_feats, edge_attrs, senders, receivers, w_mlp0, w_mlp1,
                 w_mlp2):
    import ml_dtypes
    bf = ml_dtypes.bfloat16

    node_perm, out_perm = _col_perms()

    senders = np.asarray(senders).astype(np.int64)
    receivers = np.asarray(receivers).astype(np.int64)
    edge_attrs = np.asarray(edge_attrs, dtype=np.float32)
    node_feats = np.asarray(node_feats, dtype=np.float32)

    order = np.argsort(receivers, kind="stable")
    r_s = receivers[order]
    s_s = senders[order]
    ea_s = edge_attrs[order]

    bounds = np.searchsorted(r_s, np.arange(NCORES + 1) * NODES_PER_CORE)

    nlo = np.zeros((NCORES, WINDOWS), np.int64)
    nhi = np.zeros((NCORES, WINDOWS), np.int64)
    core_data = []
    for k in range(NCORES):
        a, b = bounds[k], bounds[k + 1]
        lrcv = r_s[a:b] - k * NODES_PER_CORE
        win = (lrcv >> 7).astype(np.int64)
        is_hi = s_s[a:b] >= SPLIT
        nlo[k] = np.bincount(win[~is_hi], minlength=WINDOWS)
        nhi[k] = np.bincount(win[is_hi], minlength=WINDOWS)
        core_data.append((a, b, lrcv, win, is_hi))

    meta = _build_structure(nlo, nhi)
    CL, CH, batches, chunks, groups, lo_base, hi_base, TC = meta
    TCP = TC * P

    # sigmoid-basis fit of the gate MLP
    wA, Wc, pad_val, fit_err = _fit_pwl(w_mlp0, w_mlp1, w_mlp2,
                                        edge_attrs[:, 0])

    node_bf = np.ascontiguousarray(node_feats[:, node_perm]).astype(bf)
    wA = wA.astype(bf)
    wc_dup = np.vstack([Wc, Wc]).astype(bf)
    iota_f = np.tile(
        np.arange(P, dtype=np.float32)[None, :, None], (P, 1, CGMAX)
    ).astype(bf)

    shared = {
        "node_bf": node_bf,
        "wA": wA,
        "wc_dup": wc_dup,
        "iota_f": iota_f,
    }

    in_maps = []
    for k in range(NCORES):
        a, b, lrcv, win, is_hi = core_data[k]
        # rank within (window, half)
        keys = win * 2 + is_hi
        order2 = np.argsort(keys, kind="stable")
        ranks = np.empty(b - a, np.int64)
        sk = keys[order2]
        starts = np.r_[0, np.flatnonzero(sk[1:] != sk[:-1]) + 1]
        run_id = np.cumsum(np.r_[0, sk[1:] != sk[:-1]])
        ranks[order2] = np.arange(b - a) - starts[run_id]
        base = np.where(is_hi, hi_base[win], lo_base[win]) * P
        dst = base + ranks

        sp = np.zeros(TCP, np.int64)
        asn = np.zeros(TCP, bool)
        rp = np.zeros(TCP, np.float32)
        eap = np.zeros((TCP, 3), np.float32)
        e0 = np.full(TCP, pad_val, np.float32)
        sp[dst] = s_s[a:b]
        asn[dst] = True
        rp[dst] = (lrcv & 127).astype(np.float32)
        eap[dst] = ea_s[a:b, 1:4]
        e0[dst] = ea_s[a:b, 0]

        m = {
            "ea4": np.concatenate(
                [eap.reshape(TC, P, 3).transpose(1, 0, 2),
                 np.zeros((P, TC, 1), np.float32)], axis=2).astype(bf),
            "rcv_f": np.ascontiguousarray(rp.reshape(TC, P).T).astype(bf),
            "ea0r2": np.stack(
                [e0, np.ones(TCP, np.float32)]).astype(bf),
            **shared,
        }
        for bi, bt in enumerate(batches):
            lo_sl = slice(bt.coff * P, (bt.coff + bt.CLb) * P)
            hi_sl = slice((bt.coff + bt.CLb) * P, (bt.coff + bt.Cb) * P)
            lo_vals = np.where(asn[lo_sl], sp[lo_sl], 0).astype(np.int16)
            hi_vals = np.where(asn[hi_sl], sp[hi_sl] - SPLIT, 0).astype(np.int16)
            m[f"lo_i{bi}"] = (_wrap_idx(lo_vals) if bt.CLb else
                              np.zeros((P, 1), np.int16))
            m[f"hi_i{bi}"] = (_wrap_idx(hi_vals) if bt.CHb else
                              np.zeros((P, 1), np.int16))
        in_maps.append(m)
    return in_maps, meta, out_perm, fit_err


def kernel(node_feats, edge_attrs, senders, receivers, w_mlp0, w_mlp1, w_mlp2):
    from concourse import bass_utils

    in_maps, meta, out_perm, _ = _prep_inputs(
        node_feats, edge_attrs, senders, receivers, w_mlp0, w_mlp1, w_mlp2)
    CL, CH = meta[0], meta[1]

    key = (tuple(CL), tuple(CH))
    if key not in _CACHE:
        _CACHE[key] = _build_program(meta)
    nc = _CACHE[key]

    res = bass_utils.run_bass_kernel_spmd(
        nc, in_maps, core_ids=list(range(NCORES)))

    out = np.concatenate(
        [np.asarray(res.results[k]["out"][:NODES_PER_CORE], dtype=np.float32)
         for k in range(NCORES)], axis=0)
    return np.ascontiguousarray(out[:, out_perm])
